# revision 4
# baseline (speedup 1.0000x reference)
"""Trainium2 Bass kernel for nn_Cluster_46574625358249 (vq_codebook), v3.

Sharding: 4 fold-regions x 2 spatial-column-halves = 8 cores.

Host does index-only prep: region compaction, bilinear tap gather of raw x,
the tiny 65x64 feat/center convs (needed anyway for the l2 normalization
scalars), and 12-bit hi/lo splits of feat and alpha*l2norm(centers) STACKED
along the contraction axis:
    f2  = [fh; fl]  [128, K]     ch2 = [ch; ch]  [128, M]   cl2 = [cl; cl]
The device sim GEMM is then two exact f32r matmuls per chunk:
    az = f2^T@ch2 + f2^T@cl2 = (fh+fl)*(ch+cl) = feat*cnhat   (exact:
12-bit operands pass through f32r's input truncation unchanged; products
are 24-bit, accumulated in fp32 PSUM).

Device per core (k-half of a region; K=1792 padded columns, M centers):
  B1 per kt (pipelined across PE/DVE/ACT):
    az [128,M] (psum, split 1536|rest):  10 matmuls, one lhsT = f2[:,kt]
    mxa/mxb/rm = reduce_max chain        (DVE)
    W[kt] = Sign(rm - az) in {0,+1} bf16 (ACT; sign(0)=0 -> winner 0)
    colval[kt] = sigmoid(rm*invx+beta)   (ACT, table loaded once at kt0)
    vt[kt] = xc_bf_kt^T @ Wvbp_bf        (bf16; col 64 = kmask)
    rhsp[kt] = vt[kt]*colval[kt] bf16    (DVE)
  B2 kt-outer into 5 persistent psum banks; last chunk carries a ones
  column whose accumulated value is rowsum(rhsp):
    agg_c[c] += rhsp_kt^T @ W_kt         (bf16 matmul, fp32 psum)
    drain: agg = rs - agg_c  (DVE fused (x*-1)+rs; W is one-minus-onehot)
  agg row 64 = denominator (rhsp col 64 = colval).

Host combine: per region sum the two half partials, out = (agg + vc)/
(denom + 1), tiny 64x64 projection, scatter rows back to point order.
"""

import numpy as np
import ml_dtypes

FOLD_H = 2
FOLD_W = 2
SIZE_W = 1296.0
SIZE_H = 384.0
RH, RW = 32, 108          # folded region map H, W
HW = RH * RW              # 3456
K_HALF = HW // 2          # 1728
K_PAD = 1792              # 14*128
NKT = K_PAD // 128        # 14
M_PAD_DEFAULT = 2176      # 17*128 >= max region count + 1 ghost
R = FOLD_H * FOLD_W
N_CORES = 8

_BUILT = {}
_LAST_IN_MAPS = None


def _build(m_pad, mw):
    """mw = live center columns (max cnt + 1 ghost, rounded to x4 for f32r's
    even-N matmul restriction); columns beyond are redundant ghost copies the
    host never reads."""
    from concourse import bacc, mybir
    from concourse.tile import TileContext

    f32 = mybir.dt.float32
    f32r = mybir.dt.float32r
    bf16 = mybir.dt.bfloat16
    m_a = 1536                     # az split: [0,1536) + [1536,mw)
    m_b = mw - m_a
    mp1 = mw + 1                   # w_all per-kt stride (ones col at mw)
    a_chunks = [(0, 512), (512, 512), (1024, 512)]
    b_chunks = [(c, min(512, m_b - c)) for c in range(0, m_b, 512)]
    m_chunks = [(c, min(512, m_pad - c)) for c in range(0, m_pad, 512)]
    k_chunks = [(c, min(512, K_PAD - c)) for c in range(0, K_PAD, 512)]
    g_chunks = [(c, min(512, mw - c)) for c in range(0, mw, 512)]

    nc = bacc.Bacc(None, target_bir_lowering=False)
    f2_d = nc.dram_tensor("f2", [128, K_PAD], f32r, kind="ExternalInput")
    ch_d = nc.dram_tensor("ch2", [128, m_pad], f32r, kind="ExternalInput")
    cl_d = nc.dram_tensor("cl2", [128, m_pad], f32r, kind="ExternalInput")
    vt_d = nc.dram_tensor("vt_bf", [128, NKT * 65], bf16, kind="ExternalInput")
    invx_d = nc.dram_tensor("invx", [128, NKT], f32, kind="ExternalInput")
    beta_d = nc.dram_tensor("beta128", [128, 1], f32, kind="ExternalInput")
    agg_out = nc.dram_tensor("agg_out", [65, mw + 1], f32, kind="ExternalOutput")

    Sign = mybir.ActivationFunctionType.Sign
    Sig = mybir.ActivationFunctionType.Sigmoid
    X = mybir.AxisListType.X
    MUL = mybir.AluOpType.mult
    ADD = mybir.AluOpType.add

    with TileContext(nc) as tc:
        with tc.tile_pool(name="big", bufs=1) as big:
            f2 = big.tile([128, K_PAD], f32r)
            ch2 = big.tile([128, m_pad], f32r)
            cl2 = big.tile([128, m_pad], f32r)
            invx = big.tile([128, NKT], f32)
            beta = big.tile([128, 1], f32)
            vt = big.tile([128, NKT * 65], bf16)
            w_all = big.tile([128, NKT * mp1], bf16)
            rhsp = big.tile([128, NKT * 65], bf16)
            rm_all = big.tile([128, NKT], f32)
            agg_sb = big.tile([65, mw + 1], f32)
            sgarg = big.tile([128, NKT], f32)
            colval = big.tile([128, NKT], f32)

            # --- DMA: sync (SP) carries the c-side feed; ACT carries f2
            # and the value-path inputs
            for off, w in m_chunks:
                nc.sync.dma_start(out=ch2[:, off:off + w],
                                  in_=ch_d[:, off:off + w])
                nc.sync.dma_start(out=cl2[:, off:off + w],
                                  in_=cl_d[:, off:off + w])
            for off, w in k_chunks:
                nc.scalar.dma_start(out=f2[:, off:off + w],
                                    in_=f2_d[:, off:off + w])
            nc.scalar.dma_start(out=vt[:], in_=vt_d[:, :])
            nc.scalar.dma_start(out=invx[:], in_=invx_d[:, :])
            nc.scalar.dma_start(out=beta[:], in_=beta_d[:, :])
            # ones columns for the rowsum trick
            for kt in range(NKT):
                nc.gpsimd.memset(w_all[:, kt * mp1 + mw:kt * mp1 + mp1], 1.0)

            # ---------------- B1 ----------------
            with tc.tile_pool(name="az0", bufs=2, space="PSUM") as az0p, \
                 tc.tile_pool(name="az1", bufs=2, space="PSUM") as az1p, \
                 tc.tile_pool(name="az2", bufs=2, space="PSUM") as az2p, \
                 tc.tile_pool(name="azb", bufs=1, space="PSUM") as azbp, \
                 tc.tile_pool(name="smB", bufs=2) as smB:
                for kt in range(NKT):
                    ksl = slice(kt * 128, (kt + 1) * 128)
                    base = kt * mp1
                    lhs = f2[:, ksl]
                    at = [az0p.tile([128, 512], f32, tag="az0", name="az0t"),
                          az1p.tile([128, 512], f32, tag="az1", name="az1t"),
                          az2p.tile([128, 512], f32, tag="az2", name="az2t")]
                    azb = azbp.tile([128, m_b], f32, tag="azb")
                    # one lhsT for all 10 matmuls; ch2 passes (start) then
                    # cl2 passes (stop); no back-to-back same-region pairs
                    for c, (off, w) in enumerate(a_chunks):
                        nc.tensor.matmul(out=at[c][:, :w], lhsT=lhs,
                                         rhs=ch2[:, off:off + w],
                                         start=True, stop=False)
                    for off, w in b_chunks:
                        nc.tensor.matmul(out=azb[:, off:off + w], lhsT=lhs,
                                         rhs=ch2[:, m_a + off:m_a + off + w],
                                         start=True, stop=False)
                    for c, (off, w) in enumerate(a_chunks):
                        nc.tensor.matmul(out=at[c][:, :w], lhsT=lhs,
                                         rhs=cl2[:, off:off + w],
                                         start=False, stop=True)
                    for off, w in b_chunks:
                        nc.tensor.matmul(out=azb[:, off:off + w], lhsT=lhs,
                                         rhs=cl2[:, m_a + off:m_a + off + w],
                                         start=False, stop=True)
                    # per-chunk maxes overlap the later az matmuls
                    mx = smB.tile([128, 4], f32, tag="mx")
                    for c in range(3):
                        nc.vector.reduce_max(out=mx[:, c:c + 1], in_=at[c][:],
                                             axis=X)
                    nc.vector.reduce_max(out=mx[:, 3:4], in_=azb[:], axis=X)
                    nc.vector.reduce_max(out=rm_all[:, kt:kt + 1], in_=mx[:],
                                         axis=X)
                    # W = Sign(rm - az): winner 0, loser +1; smallest-lag
                    # tile (b) first so buffers release earliest
                    nc.scalar.activation(out=w_all[:, base + m_a:base + mw],
                                         in_=azb[:], func=Sign, scale=-1.0,
                                         bias=rm_all[:, kt:kt + 1])
                    for c, (off, w) in enumerate(a_chunks):
                        nc.scalar.activation(
                            out=w_all[:, base + off:base + off + w],
                            in_=at[c][:], func=Sign, scale=-1.0,
                            bias=rm_all[:, kt:kt + 1])
                    if kt == 0:
                        # preload the sigmoid ACT table while B1 runs
                        nc.scalar.activation(out=sgarg[:, 0:1],
                                             in_=rm_all[:, 0:1], func=Sig,
                                             bias=beta[:])

            # batched colval: one mult + one sigmoid for all kt
            nc.vector.tensor_tensor(out=sgarg[:], in0=rm_all[:], in1=invx[:],
                                    op=MUL)
            nc.scalar.activation(out=colval[:], in_=sgarg[:], func=Sig,
                                 bias=beta[:])
            for kt in range(NKT):
                nc.vector.tensor_scalar_mul(
                    out=rhsp[:, kt * 65:(kt + 1) * 65],
                    in0=vt[:, kt * 65:(kt + 1) * 65],
                    scalar1=colval[:, kt:kt + 1])

            # ---------------- B2: aggregation, kt-outer ----------------
            nw = len(g_chunks)
            with tc.tile_pool(name="agg2", bufs=1, space="PSUM") as aggp:
                aggc = [aggp.tile([65, 512], f32, tag=f"agg{c}", name=f"aggc{c}")
                        for c in range(nw)]
                for kt in range(NKT):
                    for c, (off, w) in enumerate(g_chunks):
                        if c == nw - 1:
                            w = w + 1          # ones column -> rowsum
                        nc.tensor.matmul(
                            out=aggc[c][:, :w],
                            lhsT=rhsp[:, kt * 65:(kt + 1) * 65],
                            rhs=w_all[:, kt * mp1 + off:kt * mp1 + off + w],
                            start=(kt == 0), stop=(kt == NKT - 1))
                # raw aggS (incl the rowsum column) out via a plain copy;
                # the host computes agg_true = rowsum - aggS
                for c, (off, w) in enumerate(g_chunks):
                    if c == nw - 1:
                        w = w + 1
                    if c % 2 == 0:
                        nc.scalar.copy(out=agg_sb[:, off:off + w],
                                       in_=aggc[c][:, :w])
                    else:
                        nc.vector.tensor_copy(out=agg_sb[:, off:off + w],
                                              in_=aggc[c][:, :w])
                    if c % 2 == 0:
                        nc.sync.dma_start(out=agg_out[:, off:off + w],
                                          in_=agg_sb[:, off:off + w])
                    else:
                        nc.scalar.dma_start(out=agg_out[:, off:off + w],
                                            in_=agg_sb[:, off:off + w])
    nc.compile()
    return nc


def _f32(x):
    return np.ascontiguousarray(np.asarray(x), dtype=np.float32)


def _split12(v):
    u = np.ascontiguousarray(v, dtype=np.float32).view(np.uint32)
    hi = (u & np.uint32(0xFFFFF000)).view(np.float32)
    lo = (v - hi).astype(np.float32)
    return hi, lo


def _region_indices(points):
    rh = np.float32(SIZE_H / FOLD_H)
    rw = np.float32(SIZE_W / FOLD_W)
    px, py = points[:, 0], points[:, 1]
    idxs = []
    for i in range(FOLD_H):
        for j in range(FOLD_W):
            m = (py > rh * i) & (py <= rh * (i + 1)) & \
                (px > rw * j) & (px <= rw * (j + 1))
            idxs.append(np.nonzero(m)[0])
    return idxs


def _bilinear_taps(pts):
    one = np.float32(1.0)
    gridx = pts[:, 0] / np.float32(SIZE_W - 1.0) * np.float32(2.0) - one
    gridy = pts[:, 1] / np.float32(SIZE_H - 1.0) * np.float32(2.0) - one
    gx = (gridx + one) * np.float32(RW * 0.5) - np.float32(0.5)
    gy = (gridy + one) * np.float32(RH * 0.5) - np.float32(0.5)
    x0 = np.floor(gx)
    y0 = np.floor(gy)
    wx = (gx - x0).astype(np.float32)
    wy = (gy - y0).astype(np.float32)
    x0i = np.clip(x0, 0, RW - 1).astype(np.int32)
    x1i = np.clip(x0 + 1.0, 0, RW - 1).astype(np.int32)
    y0i = np.clip(y0, 0, RH - 1).astype(np.int32)
    y1i = np.clip(y0 + 1.0, 0, RH - 1).astype(np.int32)
    taps = np.stack([y0i * RW + x0i, y0i * RW + x1i,
                     y1i * RW + x0i, y1i * RW + x1i], axis=1)
    w = np.stack([(one - wx) * (one - wy), wx * (one - wy),
                  (one - wx) * wy, wx * wy], axis=1).astype(np.float32)
    # Clamp-collapsed points (all 4 taps at one pixel, e.g. ghost slots and
    # border points): use weight (1,0,0,0) so those columns are bit-identical
    # to the ghost column, making argmax ties exact and deterministic.
    collapsed = (x0i == x1i) & (y0i == y1i)
    w[collapsed] = np.array([1.0, 0.0, 0.0, 0.0], np.float32)
    return taps, w


def kernel(points, x, W_f, b_f, W_v, b_v, W_proj, b_proj, sim_alpha, sim_beta):
    from concourse.bass_utils import run_bass_kernel_spmd

    points = _f32(points)[0]
    x = _f32(x)[0]
    W_f, b_f = _f32(W_f), _f32(b_f)
    W_v, b_v = _f32(W_v), _f32(b_v)
    W_proj, b_proj = _f32(W_proj), _f32(b_proj)
    alpha = _f32(sim_alpha).reshape(-1)[0]
    beta = _f32(sim_beta).reshape(-1)[0]
    N = points.shape[0]

    idxs = _region_indices(points)
    cnts = [len(ix) for ix in idxs]
    m_pad = M_PAD_DEFAULT
    need = max(cnts) + 1
    if need > m_pad:
        m_pad = ((need + 127) // 128) * 128
    mw = min(((need + 3) // 4) * 4, m_pad)   # live cols (f32r needs even N)

    Wfb = np.concatenate([W_f.T, b_f[None, :]], axis=0).astype(np.float32)
    Wvb = np.concatenate([W_v.T, b_v[None, :]], axis=0).astype(np.float32)
    e_one = np.zeros((65, 1), np.float32)
    e_one[64, 0] = 1.0
    Wvbp = np.ascontiguousarray(np.concatenate([Wvb, e_one], axis=1))
    beta128 = np.full((128, 1), beta, np.float32)
    bfd = ml_dtypes.bfloat16

    in_maps = []
    vcts = []
    for r in range(R):
        i, j = divmod(r, FOLD_W)
        xr = x[:, i * RH:(i + 1) * RH, j * RW:(j + 1) * RW].reshape(64, HW)
        idx_r = idxs[r]
        cnt = len(idx_r)
        pts_r = np.zeros((m_pad, 2), np.float32)
        pts_r[:cnt] = points[idx_r]
        taps, w = _bilinear_taps(pts_r)
        g = xr[:, taps]                                    # [64, m_pad, 4]
        xg = np.einsum("cmt,mt->cm", g, w).astype(np.float32)
        xg1 = np.ascontiguousarray(
            np.concatenate([xg, np.ones((1, m_pad), np.float32)], axis=0))
        centers = (xg1.T @ Wfb).astype(np.float32)         # [m_pad, 64]
        nc2 = (centers * centers).sum(axis=1, dtype=np.float32)
        s = ((np.float32(1.0) / np.sqrt(nc2 + np.float32(1e-12))) * alpha
             ).astype(np.float32)
        cnhatT = np.ascontiguousarray((centers * s[:, None]).T)  # [64, m_pad]
        ch, cl = _split12(cnhatT)
        ch2 = np.ascontiguousarray(np.concatenate([ch, ch], axis=0))
        cl2 = np.ascontiguousarray(np.concatenate([cl, cl], axis=0))
        vcT_host = np.ascontiguousarray((xg1.T @ Wvb).T)          # [64, m_pad]
        featT = (np.concatenate([xr, np.ones((1, HW), np.float32)], axis=0).T
                 @ Wfb).astype(np.float32)                 # [HW, 64]
        nfx = (featT * featT).sum(axis=1, dtype=np.float32)
        invx_full = (np.float32(1.0) / np.sqrt(nfx + np.float32(1e-12))
                     ).astype(np.float32)
        featTT = featT.T                                   # [64, HW]
        vcts.append(vcT_host)
        for h in range(2):
            ffull = np.zeros((64, K_PAD), np.float32)
            ffull[:, :K_HALF] = featTT[:, h * K_HALF:(h + 1) * K_HALF]
            fhh, fll = _split12(ffull)
            f2 = np.ascontiguousarray(np.concatenate([fhh, fll], axis=0))
            xc = np.zeros((65, K_PAD), np.float32)
            xc[:64, :K_HALF] = xr[:, h * K_HALF:(h + 1) * K_HALF]
            xc[64, :K_HALF] = 1.0
            vt_host = np.empty((128, NKT * 65), np.float32)
            for kt in range(NKT):
                vt_host[:, kt * 65:(kt + 1) * 65] = \
                    xc[:, kt * 128:(kt + 1) * 128].T @ Wvbp
            iv = np.full((K_PAD,), 1e6, np.float32)
            iv[:K_HALF] = invx_full[h * K_HALF:(h + 1) * K_HALF]
            invx = np.ascontiguousarray(iv.reshape(NKT, 128).T)   # [128, NKT]
            in_maps.append({
                "f2": f2, "ch2": ch2, "cl2": cl2,
                "vt_bf": np.ascontiguousarray(vt_host.astype(bfd)),
                "invx": invx, "beta128": beta128,
            })

    global _LAST_IN_MAPS
    _LAST_IN_MAPS = in_maps
    key = (m_pad, mw)
    if key not in _BUILT:
        _BUILT[key] = _build(m_pad, mw)
    res = run_bass_kernel_spmd(_BUILT[key], in_maps,
                               core_ids=list(range(N_CORES)))
    results = res.results

    out = np.zeros((64, N), np.float32)
    for r in range(R):
        araw = results[2 * r]["agg_out"] + results[2 * r + 1]["agg_out"]
        a = araw[:, -1:] - araw[:, :-1]          # agg_true = rowsum - aggS
        vcT = vcts[r]                            # host-computed value centers
        idx_r = idxs[r]
        cnt = len(idx_r)
        ort = (a[:64, :cnt] + vcT[:, :cnt]) / \
            (a[64, :cnt] + np.float32(1.0))[None, :]
        proj = W_proj @ ort + b_proj[:, None]
        mask = np.any(ort != 0.0, axis=0)
        out[:, idx_r] = proj * mask[None, :]
    return out[None, :, None, :]
